# revision 1
# baseline (speedup 1.0000x reference)
"""Trainium2 Bass kernel for nn_MHSA_CGLU (PSA attention + Convolutional GLU block).

Sharding: data-parallel over batch (B=8) across 8 NeuronCores, one batch each.
All activations live in [channels, N=H*W] layout (channels on SBUF partitions).
All matmuls run in float32r (TF32-like, 1 cycle/row).
"""

import ml_dtypes
import numpy as np

import concourse.bass as bass  # noqa: F401
import concourse.mybir as mybir
import concourse.tile as tile
from concourse import bacc
from concourse.bass_utils import run_bass_kernel_spmd

F32 = mybir.dt.float32
F32R = mybir.dt.float32r
BF16 = mybir.dt.bfloat16
AF = mybir.ActivationFunctionType
OP = mybir.AluOpType

EPS = 1e-5
NH, KD, HD = 8, 16, 32
C, N, HH, WW = 256, 1024, 32, 32
HID = 170
SCALE = KD ** -0.5


# --------------------------------------------------------------------------
# Host-side parameter folding
# --------------------------------------------------------------------------

def _bn_fold(p):
    g, b, m, v = [np.asarray(a, np.float64) for a in p]
    s = g / np.sqrt(v + EPS)
    return s, b - s * m


def fold_consts(inp):
    f64 = lambda a: np.asarray(a, np.float64)
    ln1_g, ln1_b = f64(inp["ln1_g"]), f64(inp["ln1_b"])
    ln2_g, ln2_b = f64(inp["ln2_g"]), f64(inp["ln2_b"])

    # qkv conv + BN, with LN1 affine folded in.
    s_qkv, b_qkv = _bn_fold(inp["qkv_bn"])
    Wq = s_qkv[:, None] * f64(inp["qkv_w"])          # [512, 256]
    bq = b_qkv.copy()                                 # [512]
    bq += Wq @ ln1_b
    Wq = Wq * ln1_g[None, :]

    # per-head rows in reference order
    q_rows = np.concatenate([np.arange(64 * h, 64 * h + 16) for h in range(NH)])
    k_rows = q_rows + 16
    v_rows = np.concatenate([np.arange(64 * h + 32, 64 * h + 64) for h in range(NH)])
    Wq_q, bq_q = Wq[q_rows] * SCALE, bq[q_rows] * SCALE   # fold softmax scale into q
    Wq_k, bq_k = Wq[k_rows], bq[k_rows]
    Wq_v, bq_v = Wq[v_rows], bq[v_rows]                   # v bias deferred (see below)

    # qkv conv weight layout: 8 M-tiles of 128 rows.  Matmul operands may
    # only start at partitions {0, 32, 64}, so 3 heads per tile:
    #   tiles 0..2: q head h at tile h//3, partitions 32*(h%3)..+15 (rest zero)
    #   tiles 3..5: k likewise;  tiles 6,7: v rows 0..127 / 128..255
    Wfull = np.zeros((8 * 128, 256))
    bias_qk = np.zeros(768)
    for h in range(NH):
        T, j = divmod(h, 3)
        Wfull[T * 128 + 32 * j: T * 128 + 32 * j + 16] = Wq_q[16 * h: 16 * h + 16]
        bias_qk[T * 128 + 32 * j: T * 128 + 32 * j + 16] = bq_q[16 * h: 16 * h + 16]
        Wfull[(3 + T) * 128 + 32 * j: (3 + T) * 128 + 32 * j + 16] = Wq_k[16 * h: 16 * h + 16]
        bias_qk[(3 + T) * 128 + 32 * j: (3 + T) * 128 + 32 * j + 16] = bq_k[16 * h: 16 * h + 16]
    Wfull[6 * 128: 8 * 128] = Wq_v
    # SBUF layout [part(cin%128), ksub(cin//128), mcol]
    wqkvT = np.ascontiguousarray(
        Wfull.T.reshape(2, 128, 1024).transpose(1, 0, 2))  # [128, 2, 1024]

    # v^T conv (for attention o-matmul lhsT): [n, 33h+d] cols; col 33h+32 is
    # the ones column (zero weight; ones added via a separate rank-1 matmul).
    WvT = np.zeros((256, 264))
    for h in range(NH):
        WvT[:, 33 * h: 33 * h + 32] = Wq_v[32 * h: 32 * h + 32].T
    wvT = np.ascontiguousarray(WvT.reshape(2, 128, 264).transpose(1, 0, 2))
    onescol264 = np.zeros((1, 264))
    onescol264[0, 32::33] = 1.0

    # pe branch: dwconv3x3(v) + BN  (BN scale folded into taps)
    s_pe, b_pe = _bn_fold(inp["pe_bn"])
    taps_pe = s_pe[:, None, None] * f64(inp["pe_w"])[:, 0]     # [256, 3, 3]
    # o2 = o_norm + bq_v (softmax weights sum to 1) + pe_out bias; both
    # per-channel constants, folded through the proj conv into its bias.
    bfold_pe = b_pe + bq_v

    # proj conv + BN
    s_pr, b_pr = _bn_fold(inp["proj_bn"])
    Wpr = s_pr[:, None] * f64(inp["proj_w"])
    bias_proj = b_pr + Wpr @ bfold_pe
    wprojT = np.ascontiguousarray(Wpr.T.reshape(2, 128, 256).transpose(1, 0, 2))

    # fc1 with LN2 affine folded; M-tiles A1(128) A2(42) G1(128) G2(42)
    W1 = f64(inp["fc1_w"])
    b1 = f64(inp["fc1_b"]) + W1 @ ln2_b
    W1 = W1 * ln2_g[None, :]
    W1cols = np.zeros((256, 512))
    b1cols = np.zeros(512)
    W1cols[:, 0:128] = W1[0:128].T;        b1cols[0:128] = b1[0:128]
    W1cols[:, 128:170] = W1[128:170].T;    b1cols[128:170] = b1[128:170]
    W1cols[:, 256:384] = W1[170:298].T;    b1cols[256:384] = b1[170:298]
    W1cols[:, 384:426] = W1[298:340].T;    b1cols[384:426] = b1[298:340]
    wfc1T = np.ascontiguousarray(W1cols.reshape(2, 128, 512).transpose(1, 0, 2))

    # GLU dwconv taps + bias
    taps_dw = f64(inp["dw_w"])[:, 0]                            # [170, 3, 3]
    b_dw = f64(inp["dw_b"])

    # fc2; K-tiles = a rows 0..127 / 128..169
    W2 = f64(inp["fc2_w"])                                      # [256, 170]
    W2T = np.zeros((2, 128, 256))
    W2T[0] = W2[:, 0:128].T
    W2T[1, 0:42] = W2[:, 128:170].T
    wfc2T = np.ascontiguousarray(W2T.transpose(1, 0, 2))        # [128, 2, 256]
    bfin = f64(inp["fc2_b"]) + ln2_b                            # fc2_b + b2

    # small per-partition vectors [128, 2, 8]: 0 bv, 1 bfold_pe, 2 b_dw, 3 g2, 4 bfin
    pvec = np.zeros((128, 2, 8))
    pvec[:, 0, 0], pvec[:, 1, 0] = bq_v[0:128], bq_v[128:256]
    pvec[:, 0, 1], pvec[:, 1, 1] = bfold_pe[0:128], bfold_pe[128:256]
    pvec[0:128, 0, 2] = b_dw[0:128]
    pvec[0:42, 1, 2] = b_dw[128:170]
    pvec[:, 0, 3], pvec[:, 1, 3] = ln2_g[0:128], ln2_g[128:256]
    pvec[:, 0, 4], pvec[:, 1, 4] = bfin[0:128], bfin[128:256]

    # tap scalar columns [128, 2, 9] (tap index = 3*dy + dx)
    tpe = np.zeros((128, 2, 9))
    tdw = np.zeros((128, 2, 9))
    for t in range(9):
        dy, dx = divmod(t, 3)
        tpe[:, 0, t] = taps_pe[0:128, dy, dx]
        tpe[:, 1, t] = taps_pe[128:256, dy, dx]
        tdw[0:128, 0, t] = taps_dw[0:128, dy, dx]
        tdw[0:42, 1, t] = taps_dw[128:170, dy, dx]

    ind = np.zeros((8, 256))
    for h in range(NH):
        ind[h, 32 * h: 32 * h + 32] = 1.0

    f32 = lambda a: np.ascontiguousarray(a, dtype=np.float32)
    bf16 = lambda a: np.ascontiguousarray(a, dtype=ml_dtypes.bfloat16)
    return {
        "wqkvT": bf16(wqkvT), "bias_qk": bf16(bias_qk.reshape(1, 768)),
        "wvT": bf16(wvT), "onescol264": bf16(onescol264),
        "wprojT": bf16(wprojT), "bias_proj": bf16(bias_proj.reshape(1, 256)),
        "wfc1T": bf16(wfc1T), "bias_fc1": bf16(b1cols.reshape(1, 512)),
        "wfc2T": bf16(wfc2T),
        "pvec": f32(pvec), "taps_pe": f32(tpe), "taps_dw": f32(tdw),
        "ind": f32(ind),
        "ones_row": bf16(np.ones((1, 512))),
        "ones_rowf": f32(np.ones((1, 512))),
        "epsrow": f32(np.full((1, 1), EPS)),
        "ones_col": f32(np.ones((128, 1))),
        "id128": f32(np.eye(128)),
    }


# --------------------------------------------------------------------------
# Device program (one core, one batch)
# --------------------------------------------------------------------------

def _ln(nc, psA, psB, big, work, x_tiles, consts, out_dtype=BF16):
    """LayerNorm over channels (partition dim). Returns z tiles (2x [128,1024]),
    z = (x - mu) * rstd  (affine folded into downstream weights)."""
    ones_col, ones_row = consts["ones_col"], consts["ones_row"]
    xsq = [big.tile([128, N], F32R, tag="pt", name=f"xsq{t}") for t in range(2)]
    for t in range(2):
        nc.vector.tensor_tensor(xsq[t][:], x_tiles[t][:], x_tiles[t][:], OP.mult)
    sum_ps = psB.tile([1, N], F32, tag="psB", name="psB")
    ssq_ps = psB.tile([1, N], F32, tag="psB", name="psB")
    for c in range(2):
        sl = slice(c * 512, (c + 1) * 512)
        for t in range(2):
            nc.tensor.matmul(sum_ps[:, sl], ones_col[:], x_tiles[t][:, sl],
                             start=(t == 0), stop=(t == 1))
        for t in range(2):
            nc.tensor.matmul(ssq_ps[:, sl], ones_col[:], xsq[t][:, sl],
                             start=(t == 0), stop=(t == 1))
    # row math (only one PSUM operand allowed per DVE op)
    msb = big.tile([1, N], F32, tag="pt", name="ln_msb")
    esq = big.tile([1, N], F32, tag="pt", name="ln_esq")
    t1 = big.tile([1, N], F32, tag="pt", name="ln_t1")
    A = big.tile([1, N], F32R, tag="pt", name="ln_A")
    Br = big.tile([1, N], F32R, tag="pt", name="ln_B")
    nc.vector.tensor_scalar(msb[:], sum_ps[:], -1.0 / C, None, OP.mult)  # -mu
    nc.vector.tensor_scalar(esq[:], ssq_ps[:], 1.0 / C, None, OP.mult)   # E[x^2]
    nc.vector.tensor_tensor(t1[:], msb[:], msb[:], OP.mult)              # mu^2
    nc.vector.tensor_tensor(t1[:], esq[:], t1[:], OP.subtract)           # var
    nc.scalar.activation(t1[:], t1[:], AF.Ln, bias=consts["epsrow"][:])
    nc.scalar.activation(A[:], t1[:], AF.Exp, scale=-0.5)                # rstd
    nc.vector.tensor_tensor(Br[:], msb[:], A[:], OP.mult)                # -mu*rstd
    # broadcast A,B rows to 128 partitions via K=1 matmuls
    Abc = psA.tile([128, N], F32, tag="psA", name="psA")
    Bbc = psA.tile([128, N], F32, tag="psA", name="psA")
    ones_rowf = consts["ones_rowf"]
    for c in range(2):
        sl = slice(c * 512, (c + 1) * 512)
        nc.tensor.matmul(Abc[:, sl], ones_rowf[:, 0:128], A[:, sl],
                         start=True, stop=True)
        nc.tensor.matmul(Bbc[:, sl], ones_rowf[:, 0:128], Br[:, sl],
                         start=True, stop=True)
    z = [work.tile([128, N], out_dtype, tag=f"z{t}", name=f"z{t}") for t in range(2)]
    for t in range(2):
        nc.vector.tensor_tensor(z[t][:], x_tiles[t][:], Abc[:], OP.mult)
        nc.vector.tensor_tensor(z[t][:], z[t][:], Bbc[:], OP.add)
    return z


def _dwconv9(nc, psA, pad_tiles, taps_sb, nparts, out_tag):
    """Depthwise 3x3 as 9 diagonal matmuls per partition-tile per chunk.
    pad_tiles: list of [128, 34, 34] padded inputs. Returns psum tiles."""
    outs = []
    for t, (pad, npart) in enumerate(zip(pad_tiles, nparts)):
        ps = psA.tile([128, N], F32, tag=out_tag, name=out_tag)
        for c in range(2):
            for tap in range(9):
                dy, dx = divmod(tap, 3)
                rhs = pad[0:npart, dy + 16 * c: dy + 16 * c + 16, dx: dx + 32]
                nc.tensor.matmul(
                    ps[0:npart, c * 512:(c + 1) * 512],
                    taps_sb[t][tap][0:npart, 0:npart],
                    rhs,
                    start=(tap == 0), stop=(tap == 8))
        outs.append(ps)
    return outs


def build(num_devices=8, debug_outs=False):
    nc = bacc.Bacc("TRN2", target_bir_lowering=False, debug=False,
                   num_devices=num_devices)

    x_d = nc.dram_tensor("x", [C, N], F32R, kind="ExternalInput")
    names = [
        ("wqkvT", [128, 2, 1024], BF16), ("bias_qk", [1, 768], BF16),
        ("wvT", [128, 2, 264], BF16), ("onescol264", [1, 264], BF16),
        ("wprojT", [128, 2, 256], BF16), ("bias_proj", [1, 256], BF16),
        ("wfc1T", [128, 2, 512], BF16), ("bias_fc1", [1, 512], BF16),
        ("wfc2T", [128, 2, 256], BF16),
        ("pvec", [128, 2, 8], F32), ("taps_pe", [128, 2, 9], F32),
        ("taps_dw", [128, 2, 9], F32), ("ind", [8, 256], F32R),
        ("ones_row", [1, 512], BF16), ("ones_col", [128, 1], F32R),
        ("ones_rowf", [1, 512], F32R),
        ("epsrow", [1, 1], F32),
        ("id128", [128, 128], F32R),
    ]
    drams = {nm: nc.dram_tensor(nm, sh, dt, kind="ExternalInput")
             for nm, sh, dt in names}
    y_d = nc.dram_tensor("y", [C, N], F32, kind="ExternalOutput")
    dbg = {}
    if debug_outs:
        for nm, sh, dt in [("d_z1", [128, N], BF16), ("d_qk0", [128, N], BF16),
                           ("d_k0", [128, N], BF16), ("d_pt0", [128, N], BF16),
                           ("d_oall0", [128, N], F32), ("d_o20", [128, N], BF16),
                           ("d_xattn0", [128, N], F32), ("d_rows", [8, N], F32)]:
            dbg[nm] = nc.dram_tensor(nm, sh, dt, kind="ExternalOutput")

    with tile.TileContext(nc) as tc:
        with tc.tile_pool(name="singles", bufs=1) as singles, \
             tc.tile_pool(name="work", bufs=1) as work, \
             tc.tile_pool(name="ptp", bufs=17) as ptp, \
             tc.tile_pool(name="stg", bufs=3) as stg, \
             tc.tile_pool(name="psA", bufs=2, space="PSUM") as psA, \
             tc.tile_pool(name="psB", bufs=2, space="PSUM") as psB:

            # ---- load constants + input ----
            consts = {}
            for nm, sh, dt in names:
                t = singles.tile(sh, dt, tag=nm, name=nm)
                nc.sync.dma_start(t[:], drams[nm].ap())
                consts[nm] = t
            xt = [work.tile([128, N], F32R, tag=f"x{t}", name=f"x{t}") for t in range(2)]
            for t in range(2):
                nc.sync.dma_start(xt[t][:], x_d.ap()[t * 128:(t + 1) * 128, :])

            ones_row = consts["ones_row"]

            # build diag tap matrices on device: diag = id128 * tapcol
            diag_pe, diag_dw = [], []
            for t in range(2):
                dpe, ddw = [], []
                for tap in range(9):
                    d1 = singles.tile([128, 128], F32R, tag=f"dpe{t}_{tap}", name=f"dpe{t}_{tap}")
                    nc.vector.tensor_scalar(
                        d1[:], consts["id128"][:], consts["taps_pe"][:, t, tap:tap + 1],
                        None, OP.mult)
                    dpe.append(d1)
                    d2 = singles.tile([128, 128], F32R, tag=f"ddw{t}_{tap}", name=f"ddw{t}_{tap}")
                    nc.vector.tensor_scalar(
                        d2[:], consts["id128"][:], consts["taps_dw"][:, t, tap:tap + 1],
                        None, OP.mult)
                    ddw.append(d2)
                diag_pe.append(dpe)
                diag_dw.append(ddw)

            # ---- LN1 ----
            z1 = _ln(nc, psA, psB, ptp, work, xt, consts)

            # ---- qkv conv: M-tiles 0,1=q 2,3=k 4,5=v ----
            qk_sb = []   # 4 tiles [128, N] f32r (q0,q1,k0,k1)
            vpad = [work.tile([128, 34, 34], F32R, tag=f"pad{t}", name=f"vpad{t}") for t in range(2)]
            for t in range(2):
                nc.gpsimd.memset(vpad[t][:].bitcast(mybir.dt.uint32), 0)
            for mt in range(8):
                ps = psA.tile([128, N], F32, tag="psA", name="psA")
                has_bias = mt < 6  # q,k get conv bias; v bias deferred
                for c in range(2):
                    sl = slice(c * 512, (c + 1) * 512)
                    for kt in range(2):
                        nc.tensor.matmul(
                            ps[:, sl], consts["wqkvT"][:, kt, mt * 128:(mt + 1) * 128],
                            z1[kt][:, sl], start=(kt == 0),
                            stop=(kt == 1 and not has_bias))
                    if has_bias:
                        nc.tensor.matmul(
                            ps[:, sl], consts["bias_qk"][:, mt * 128:(mt + 1) * 128],
                            ones_row[:, 0:512], start=False, stop=True)
                if mt < 6:
                    t_sb = work.tile([128, N], BF16, tag=f"qk{mt}", name=f"qk{mt}")
                    nc.vector.tensor_copy(t_sb[:], ps[:])
                    qk_sb.append(t_sb)
                else:
                    # v: evacuate into padded dwconv input, adding v bias
                    vt = mt - 6
                    nc.vector.tensor_scalar(
                        vpad[vt][:, 1:33, 1:33], ps[:],
                        consts["pvec"][:, vt, 0:1], None, OP.add)

            # ---- v^T conv: out [n, 264] per n-tile ----
            vT_sb = []
            for nt in range(8):
                ps = psB.tile([128, 264], F32, tag="psB", name="psB")
                for kt in range(2):
                    nc.tensor.matmul(
                        ps[:], z1[kt][:, nt * 128:(nt + 1) * 128],
                        consts["wvT"][:, kt, :], start=(kt == 0), stop=False)
                nc.tensor.matmul(ps[:], ones_row[0:1, 0:128],
                                 consts["onescol264"][:], start=False, stop=True)
                t_sb = work.tile([128, 264], BF16, tag=f"vT{nt}", name=f"vT{nt}")
                nc.vector.tensor_copy(t_sb[:], ps[:])
                vT_sb.append(t_sb)

            # ---- attention ----
            o_all = [work.tile([128, N], F32R, tag=f"oall{t}", name=f"oall{t}") for t in range(2)]
            rowsums = work.tile([8, N], F32R, tag="rowsums", name="rowsums")
            for hp in range(4):
                h0, h1 = 2 * hp, 2 * hp + 1
                pts = {}
                for hh in (h0, h1):
                    T, j = divmod(hh, 3)
                    qsl = qk_sb[T][32 * j: 32 * j + 16, :]
                    ksl = qk_sb[3 + T][32 * j: 32 * j + 16, :]
                    for mt in range(8):
                        sp = psA.tile([128, N], F32, tag="psA", name="psA")
                        for c in range(2):
                            nc.tensor.matmul(
                                sp[:, c * 512:(c + 1) * 512],
                                ksl[:, mt * 128:(mt + 1) * 128],
                                qsl[:, c * 512:(c + 1) * 512],
                                start=True, stop=True)
                        pt = ptp.tile([128, N], BF16, tag="pt", name="pt")
                        nc.scalar.activation(pt[:], sp[:], AF.Exp)
                        pts[(hh, mt)] = pt
                # o-matmuls for the head pair, column-tiled 2x concurrent:
                # head h0 -> PSUM partitions 0:33, head h1 -> 64:97
                ops = psB.tile([128, N], F32, tag="psB", name="psB")
                for c in range(2):
                    sl = slice(c * 512, (c + 1) * 512)
                    for mt in range(8):
                        nc.tensor.matmul(
                            ops[0:33, sl],
                            vT_sb[mt][:, 33 * h0: 33 * h0 + 33],
                            pts[(h0, mt)][:, sl],
                            start=(mt == 0), stop=(mt == 7),
                            tile_position=(0, 0))
                        nc.tensor.matmul(
                            ops[64:97, sl],
                            vT_sb[mt][:, 33 * h1: 33 * h1 + 33],
                            pts[(h1, mt)][:, sl],
                            start=(mt == 0), stop=(mt == 7),
                            tile_position=(0, 64))
                stage = stg.tile([97, N], F32R, tag="stage", name="stage")
                nc.vector.tensor_copy(stage[:], ops[0:97, :])
                for hh, base in ((h0, 0), (h1, 64)):
                    nc.sync.dma_start(rowsums[hh:hh + 1, :], stage[base + 32: base + 33, :])
                    oT, oj = divmod(hh, 4)
                    nc.sync.dma_start(o_all[oT][32 * oj: 32 * oj + 32, :],
                                      stage[base: base + 32, :])

            if debug_outs:
                nc.sync.dma_start(dbg["d_z1"].ap(), z1[0][:])
                nc.sync.dma_start(dbg["d_qk0"].ap(), qk_sb[0][:])
                nc.sync.dma_start(dbg["d_k0"].ap(), qk_sb[3][:])
                nc.sync.dma_start(dbg["d_oall0"].ap(), o_all[0][:].bitcast(F32))
                nc.sync.dma_start(dbg["d_rows"].ap(), rowsums[:].bitcast(F32))

            # softmax denominators -> broadcast reciprocal
            recip = work.tile([8, N], F32R, tag="recip", name="recip")
            with nc.allow_low_precision(reason="f32r output feeds matmul rhs"):
                nc.vector.reciprocal(recip[:], rowsums[:])
            recipB = [psA.tile([128, N], F32, tag="psA", name="psA") for _ in range(2)]
            for t in range(2):
                for c in range(2):
                    sl = slice(c * 512, (c + 1) * 512)
                    nc.tensor.matmul(recipB[t][:, sl],
                                     consts["ind"][:, t * 128:(t + 1) * 128],
                                     recip[:, sl], start=True, stop=True)

            # ---- pe branch: dwconv3x3(v)+bn ----
            pe_ps = _dwconv9(nc, psB, vpad, diag_pe, [128, 128], "psB")

            # o2 = o_all*recipB + pe (bf16 out, feeds proj matmul)
            # (all per-channel biases folded into proj bias)
            o2 = [work.tile([128, N], BF16, tag=f"o2{t}", name=f"o2{t}") for t in range(2)]
            for t in range(2):
                nc.vector.tensor_tensor(o2[t][:], o_all[t][:], recipB[t][:], OP.mult)
                nc.vector.tensor_tensor(o2[t][:], o2[t][:], pe_ps[t][:], OP.add)

            if debug_outs:
                nc.sync.dma_start(dbg["d_o20"].ap(), o2[0][:])

            # ---- proj conv + residual (in place on x tiles) ----
            x_attn = xt
            for mt in range(2):
                ps = psA.tile([128, N], F32, tag="psA", name="psA")
                for c in range(2):
                    sl = slice(c * 512, (c + 1) * 512)
                    for kt in range(2):
                        nc.tensor.matmul(
                            ps[:, sl], consts["wprojT"][:, kt, mt * 128:(mt + 1) * 128],
                            o2[kt][:, sl], start=(kt == 0), stop=False)
                    nc.tensor.matmul(
                        ps[:, sl], consts["bias_proj"][:, mt * 128:(mt + 1) * 128],
                        ones_row[:, 0:512], start=False, stop=True)
                nc.vector.tensor_tensor(x_attn[mt][:], xt[mt][:], ps[:], OP.add)

            if debug_outs:
                nc.sync.dma_start(dbg["d_xattn0"].ap(), x_attn[0][:].bitcast(F32))

            # ---- LN2 ----
            z2 = _ln(nc, psA, psB, ptp, work, x_attn, consts)

            # ---- fc1: M-tiles A1(128) A2(42) G1(128) G2(42) ----
            apad = [work.tile([128, 34, 34], F32R, tag=f"pad{t}", name=f"apad{t}") for t in range(2)]
            for t in range(2):
                nc.gpsimd.memset(apad[t][:].bitcast(mybir.dt.uint32), 0)
            g_ps = []
            nparts = [128, 42, 128, 42]
            for mt in range(4):
                npart = nparts[mt]
                ps = psA.tile([128, N], F32, tag="psA", name="psA")
                for c in range(2):
                    sl = slice(c * 512, (c + 1) * 512)
                    for kt in range(2):
                        nc.tensor.matmul(
                            ps[0:npart, sl],
                            consts["wfc1T"][:, kt, mt * 128: mt * 128 + npart],
                            z2[kt][:, sl], start=(kt == 0), stop=False)
                    nc.tensor.matmul(
                        ps[0:npart, sl],
                        consts["bias_fc1"][:, mt * 128: mt * 128 + npart],
                        ones_row[:, 0:512], start=False, stop=True)
                if mt < 2:
                    nc.vector.tensor_copy(apad[mt][0:npart, 1:33, 1:33], ps[0:npart])
                else:
                    g_ps.append(ps)

            # GLU dwconv + gelu + gate
            da_ps = _dwconv9(nc, psB, apad, diag_dw, [128, 42], "psB")
            ag = []
            for t in range(2):
                npart = nparts[t]
                a_act = ptp.tile([128, N], F32, tag="pt", name=f"aact{t}")
                nc.scalar.activation(a_act[0:npart], da_ps[t][0:npart], AF.Gelu,
                                     bias=consts["pvec"][0:npart, t, 2:3])
                agt = ptp.tile([128, N], BF16, tag="pt", name=f"ag{t}")
                nc.vector.tensor_tensor(agt[0:npart], a_act[0:npart],
                                        g_ps[t][0:npart], OP.mult)
                ag.append(agt)

            # ---- fc2 + final residuals ----
            for mt in range(2):
                ps = psA.tile([128, N], F32, tag="psA", name="psA")
                for c in range(2):
                    sl = slice(c * 512, (c + 1) * 512)
                    for kt in range(2):
                        npart = nparts[kt]
                        nc.tensor.matmul(
                            ps[:, sl],
                            consts["wfc2T"][0:npart, kt, mt * 128:(mt + 1) * 128],
                            ag[kt][0:npart, sl], start=(kt == 0), stop=(kt == 1))
                # y = x_attn + (g2*z2 + bfin) + fc2
                xn2b = ptp.tile([128, N], F32, tag="pt", name=f"xn2b{mt}")
                nc.scalar.activation(xn2b[:], z2[mt][:], AF.Identity,
                                     bias=consts["pvec"][:, mt, 4:5],
                                     scale=consts["pvec"][:, mt, 3:4])
                nc.vector.tensor_tensor(xn2b[:], xn2b[:], x_attn[mt][:], OP.add)
                nc.vector.tensor_tensor(xn2b[:], xn2b[:], ps[:], OP.add)
                nc.sync.dma_start(y_d.ap()[mt * 128:(mt + 1) * 128, :], xn2b[:])

    nc.compile()
    return nc


_NC = None


def kernel(**inputs):
    global _NC
    consts = fold_consts(inputs)
    if _NC is None:
        _NC = build()
    x = np.asarray(inputs["x"], np.float32)
    B = x.shape[0]
    in_maps = []
    for b in range(B):
        m = dict(consts)
        m["x"] = np.ascontiguousarray(x[b].reshape(C, N))
        in_maps.append(m)
    res = run_bass_kernel_spmd(_NC, in_maps, core_ids=list(range(B)))
    out = np.stack([res.results[b]["y"].reshape(C, HH, WW) for b in range(B)])
    return out

